# revision 1
# baseline (speedup 1.0000x reference)
"""MultiHeadCrossAttention kernel for 8 Trainium2 NeuronCores.

Sharding: pure data-parallel over batch (B=8 -> 1 batch element per core).
Per-core layout strategy:
  - Activations transposed on-chip via PE transpose -> feature-major xT/keyT/valueT.
  - Projections produce qT,kT feature-major [E, L] and v token-major [L, E]
    (v stored with a ones-column per head for the softmax denominator).
  - Attention per head in transposed orientation: scoresT[k,q] = kT_h^T-slices,
    exp on ScalarE (no max subtraction: |scores*0.125| < ~4), attn_unnormT and
    denominator from one matmul using the [v_h | 1] stationary operand.
  - attn_weights (mean over heads of normalized probs) accumulated in PSUM via
    identity matmuls, transposed back to natural [q,k] at the end of each
    q-block with PE transposes.
  - out_proj + residual + LayerNorm fused per q-block of 256 rows.
All matmuls run as float32r (full PE rate at free-dim >= 256).
"""

import numpy as np
from contextlib import ExitStack

import concourse.bacc as bacc
import concourse.bass as bass
import concourse.tile as tile
from concourse import mybir
from concourse.bass_utils import run_bass_kernel_spmd
from concourse.masks import make_identity

E = 1024
H = 16
DH = 64
L = 1024
P = 128
QB = 256          # q-block size
NQB = L // QB     # 4
NKT = L // P      # 8 k-tiles
NEC = E // P      # 8 feature chunks
VS = H * (DH + 1)  # 1040 v columns per k-chunk (65 per head)
LN_EPS = 1e-5

F32 = mybir.dt.float32
F32R = mybir.dt.float32r
AF = mybir.ActivationFunctionType
OP = mybir.AluOpType


def _emit(nc, tc, io):
    x_q, k_in, v_in = io["x_q"], io["k_in"], io["v_in"]
    wqT, wkT, wvT, woT_d = io["wqT"], io["wkT"], io["wvT"], io["woT"]
    b_all, gb = io["b_all"], io["gb"]
    y_out, w_out = io["y_out"], io["w_out"]

    ctx = tc.ctx  # ExitStack from caller
    ctx.enter_context(nc.allow_low_precision("fp32r tiles"))

    const = ctx.enter_context(tc.tile_pool(name="const", bufs=1))
    persist = ctx.enter_context(tc.tile_pool(name="persist", bufs=1))
    psum_acc = ctx.enter_context(tc.tile_pool(name="psum_acc", bufs=4, space="PSUM"))
    psum_sc = ctx.enter_context(tc.tile_pool(name="psum_sc", bufs=2, space="PSUM"))
    psum_av = ctx.enter_context(tc.tile_pool(name="psum_av", bufs=2, space="PSUM"))

    ident_f = const.tile([P, P], F32)
    make_identity(nc, ident_f[:])
    ident = const.tile([P, P], F32R)
    nc.vector.tensor_copy(ident[:], ident_f[:])
    ones1f = const.tile([1, P], F32)
    nc.vector.memset(ones1f[:], 1.0)
    ones1 = const.tile([1, P], F32R)
    nc.vector.tensor_copy(ones1[:], ones1f[:])
    onesP = const.tile([P, 1], F32)
    nc.vector.memset(onesP[:], 1.0)
    eps_sb = const.tile([P, 1], F32)
    nc.vector.memset(eps_sb[:], LN_EPS)

    # biases: b_all DRAM [4, 1024] rows = bq, bk, bv, bo ; gb DRAM [2, 1024] = gamma, beta
    bqk_col = const.tile([P, 2 * NEC], F32)  # [:,0:8]=bq cols, [:,8:16]=bk cols
    for i in range(2):
        nc.sync.dma_start(
            out=bqk_col[:, NEC * i:NEC * (i + 1)],
            in_=b_all[i, :].rearrange("(m p) -> p m", p=P).bitcast(F32),
        )
    bvbo_row = const.tile([1, 2 * E], F32R)  # [0:1024]=bv, [1024:2048]=bo
    nc.sync.dma_start(out=bvbo_row[:, 0:E], in_=b_all[2:3, :])
    nc.sync.dma_start(out=bvbo_row[:, E:2 * E], in_=b_all[3:4, :])
    gb_row = const.tile([1, 2 * E], F32R)
    nc.sync.dma_start(out=gb_row[:], in_=gb[:])

    qT = persist.tile([P, NEC * L], F32R)      # [e_out, l] chunks of 128 rows
    kT = persist.tile([P, NEC * L], F32R)
    v_sb = persist.tile([P, NKT * VS], F32R)   # token-major v, 65-wide head slots
    op_ = onesP[:]
    nc.vector.tensor_copy(
        out=v_sb[:].rearrange("p (n d) -> p n d", d=DH + 1)[:, :, DH:DH + 1],
        in_=bass.AP(tensor=op_.tensor, offset=op_.offset,
                    ap=[op_.ap[0], [0, H * NKT], [0, 1]]),
    )

    # ---------------- phase 1: transposes + projections ----------------
    with tc.tile_pool(name="wt", bufs=1) as wt_pool, \
         tc.tile_pool(name="ld", bufs=3) as ld_pool, \
         tc.tile_pool(name="actT", bufs=1) as actT_pool:

        for ti, (src, w_d) in enumerate([(x_q, wqT), (k_in, wkT), (v_in, wvT)]):
            # transposed activation aT [e_in, l]
            aT = actT_pool.tile([P, NEC * L], F32R, tag="actT")
            for lc in range(NKT):
                nat = ld_pool.tile([P, E], F32R, tag="ld")
                nc.sync.dma_start(out=nat[:], in_=src[P * lc:P * (lc + 1), :])
                for ep in range(NEC // 2):
                    tp = psum_av.tile([P, 2 * P], F32R, tag="av", name=f"tp_{ti}_{lc}_{ep}")
                    for sub in range(2):
                        ec = 2 * ep + sub
                        nc.tensor.transpose(
                            tp[:, P * sub:P * (sub + 1)],
                            nat[:, P * ec:P * (ec + 1)], ident[:],
                        )
                    for sub in range(2):
                        ec = 2 * ep + sub
                        dst = aT[:, L * ec + P * lc: L * ec + P * lc + P]
                        if (lc + ep) % 2 == 0:
                            nc.scalar.copy(dst, tp[:, P * sub:P * (sub + 1)])
                        else:
                            nc.vector.tensor_copy(dst, tp[:, P * sub:P * (sub + 1)])
            wt = wt_pool.tile([P, NEC * E], F32R, tag="wt")
            for c in range(NEC):
                nc.sync.dma_start(
                    out=wt[:, E * c:E * (c + 1)], in_=w_d[P * c:P * (c + 1), :]
                )
            tiles16 = [(m, n) for m in range(NEC) for n in range(2)]
            for g in range(0, 16, 4):
                grp = tiles16[g:g + 4]
                psums = [
                    psum_acc.tile([P, 512], F32, tag="acc", name=f"ps_{ti}_{g}_{i}")
                    for i in range(len(grp))
                ]
                for c in range(NEC):
                    for i, (m, n) in enumerate(grp):
                        if ti < 2:  # qT / kT : feature-major out
                            lhsT = wt[:, E * c + P * m: E * c + P * (m + 1)]
                            rhs = aT[:, L * c + 512 * n: L * c + 512 * (n + 1)]
                        else:       # v natural
                            lhsT = aT[:, L * c + P * m: L * c + P * (m + 1)]
                            rhs = wt[:, E * c + 512 * n: E * c + 512 * (n + 1)]
                        nc.tensor.matmul(
                            psums[i][:], lhsT, rhs,
                            start=(c == 0), stop=(c == NEC - 1 and ti < 2),
                        )
                for i, (m, n) in enumerate(grp):
                    if ti < 2:
                        dst = (qT if ti == 0 else kT)[:, L * m + 512 * n: L * m + 512 * (n + 1)]
                        nc.vector.tensor_scalar_add(
                            out=dst, in0=psums[i][:],
                            scalar1=bqk_col[:, NEC * ti + m: NEC * ti + m + 1],
                        )
                    else:
                        # bias via ones-row matmul, then strided evict into head slots
                        nc.tensor.matmul(
                            psums[i][:], ones1[0:1, :],
                            bvbo_row[0:1, 512 * n:512 * (n + 1)],
                            start=False, stop=True,
                        )
                        dst = v_sb[:, VS * m + 520 * n: VS * m + 520 * (n + 1)]
                        nc.vector.tensor_copy(
                            out=dst.rearrange("p (h d) -> p h d", d=DH + 1)[:, :, 0:DH],
                            in_=psums[i][:].rearrange("p (h d) -> p h d", d=DH),
                        )

    # ---------------- phase 2: attention + out_proj + LN ----------------
    with tc.tile_pool(name="wo", bufs=1) as wo_pool, \
         tc.tile_pool(name="expT", bufs=2) as expT_pool, \
         tc.tile_pool(name="attnT", bufs=1) as attnT_pool, \
         tc.tile_pool(name="invbc", bufs=2) as invbc_pool, \
         tc.tile_pool(name="accq", bufs=1) as accq_pool, \
         tc.tile_pool(name="wnat", bufs=4) as wnat_pool, \
         tc.tile_pool(name="xqb", bufs=1) as xqb_pool, \
         tc.tile_pool(name="ysb", bufs=1) as ysb_pool, \
         tc.tile_pool(name="small", bufs=2) as small:

        woT = wo_pool.tile([P, NEC * E], F32R, tag="wo")
        for c in range(NEC):
            nc.sync.dma_start(out=woT[:, E * c:E * (c + 1)], in_=woT_d[P * c:P * (c + 1), :])
        gamma_bc = wo_pool.tile([P, E], mybir.dt.bfloat16, tag="gbc")
        beta_bc = wo_pool.tile([P, E], mybir.dt.bfloat16, tag="bbc")
        for i, dstt in enumerate([gamma_bc, beta_bc]):
            for hf in range(2):
                bcp = psum_sc.tile([P, 512], F32, tag="sc")
                nc.tensor.matmul(
                    bcp[:], ones1[0:1, :],
                    gb_row[0:1, E * i + 512 * hf: E * i + 512 * (hf + 1)],
                    start=True, stop=True,
                )
                nc.scalar.copy(dstt[:, 512 * hf:512 * (hf + 1)], bcp[:])

        for qb in range(NQB):
            q0 = QB * qb
            attnT = attnT_pool.tile([P, NEC * QB], F32R, tag="attnT")
            accs = [
                psum_acc.tile([P, 512], F32, tag="acc", name=f"acc_{qb}_{j}")
                for j in range(4)
            ]
            def head_front(h):
                hb = (h % 2) * DH
                hc = h // 2
                expT = expT_pool.tile(
                    [P, NKT * QB], F32R, tag="expT", name=f"expT_{qb}_{h}"
                )
                for j in range(4):
                    sc = psum_sc.tile([P, 512], F32, tag="sc", name=f"sc_{qb}_{h}_{j}")
                    for half in range(2):
                        kt = 2 * j + half
                        lhsT = kT[hb:hb + DH, L * hc + P * kt: L * hc + P * (kt + 1)]
                        rhs = qT[hb:hb + DH, L * hc + q0: L * hc + q0 + QB]
                        nc.tensor.matmul(
                            sc[:, QB * half:QB * (half + 1)],
                            lhsT, rhs,
                            start=True, stop=True,
                        )
                    nc.scalar.activation(
                        expT[:, 512 * j:512 * (j + 1)], sc[:], AF.Exp, scale=0.125
                    )
                return expT

            def head_tail(h, expT):
                hb = (h % 2) * DH
                hc = h // 2
                av = psum_av.tile([DH + 1, QB], F32, tag="av", name=f"av_{qb}_{h}")
                for kt in range(NKT):
                    nc.tensor.matmul(
                        av[:],
                        v_sb[:, VS * kt + (DH + 1) * h: VS * kt + (DH + 1) * (h + 1)],
                        expT[:, QB * kt:QB * (kt + 1)],
                        start=(kt == 0), stop=(kt == NKT - 1),
                    )
                inv = small.tile([1, QB], F32R, tag="inv", name=f"inv_{qb}_{h}")
                nc.vector.reciprocal(inv[:], av[DH:DH + 1, :])
                bcp = psum_sc.tile([P, QB], F32, tag="sc", name=f"bcp_{qb}_{h}")
                nc.tensor.matmul(
                    bcp[:], ones1[0:1, :], inv[:],
                    start=True, stop=True,
                )
                inv_bc = invbc_pool.tile([P, QB], F32, tag="invbc", name=f"ib_{qb}_{h}")
                nc.scalar.copy(inv_bc[:], bcp[:])
                nc.vector.tensor_tensor(
                    out=attnT[hb:hb + DH, QB * hc:QB * (hc + 1)],
                    in0=av[0:DH, :], in1=inv_bc[0:DH, :], op=OP.mult,
                )
                iap = inv_bc[:]
                bc_ap = bass.AP(
                    tensor=iap.tensor, offset=iap.offset,
                    ap=[iap.ap[0], [0, NKT], iap.ap[1]],
                )
                nc.vector.tensor_tensor(
                    out=expT[:].rearrange("p (n d) -> p n d", d=QB),
                    in0=expT[:].rearrange("p (n d) -> p n d", d=QB),
                    in1=bc_ap, op=OP.mult,
                )
                for j in range(4):
                    nc.tensor.matmul(
                        accs[j][:],
                        ident[:],
                        expT[:, 512 * j:512 * (j + 1)],
                        start=(h == 0), stop=(h == H - 1),
                    )

            for h in range(H):
                head_tail(h, head_front(h))
            # attn_weights: evict acc (mean over heads), transpose to natural
            accq = accq_pool.tile([P, NKT * QB], F32R, tag="accq")
            for j in range(4):
                nc.scalar.mul(accq[:, 512 * j:512 * (j + 1)], accs[j][:], 1.0 / H)
            for kt in range(NKT):
                for qs in range(2):
                    tp = psum_av.tile([P, P], F32R, tag="av")
                    nc.tensor.transpose(
                        tp[:], accq[:, QB * kt + P * qs: QB * kt + P * (qs + 1)], ident[:]
                    )
                    wb = wnat_pool.tile([P, P], F32, tag="wnat", name=f"wb_{qb}_{kt}_{qs}")
                    nc.vector.tensor_copy(out=wb[:], in_=tp[:])
                    nc.sync.dma_start(
                        out=w_out[q0 + P * qs: q0 + P * (qs + 1), P * kt:P * (kt + 1)],
                        in_=wb[:],
                    )
            # out_proj + residual + LN
            x_qb = xqb_pool.tile([P, 2 * E], F32R, tag="xqb")
            for qs in range(2):
                nc.sync.dma_start(
                    out=x_qb[:, E * qs:E * (qs + 1)],
                    in_=x_q[q0 + P * qs: q0 + P * (qs + 1), :],
                )
            y_sb = ysb_pool.tile([P, 2 * E], F32, tag="ysb")
            for qs in range(2):
                for eb in range(2):
                    po = psum_acc.tile([P, 512], F32, tag="acc")
                    for c in range(NEC):
                        nc.tensor.matmul(
                            po[:],
                            attnT[:, QB * c + P * qs: QB * c + P * (qs + 1)],
                            woT[:, E * c + 512 * eb: E * c + 512 * (eb + 1)],
                            start=(c == 0), stop=False,
                        )
                    nc.tensor.matmul(
                        po[:], ones1[0:1, :],
                        bvbo_row[0:1, E + 512 * eb: E + 512 * (eb + 1)],
                        start=False, stop=True,
                    )
                    nc.vector.tensor_tensor(
                        out=y_sb[:, E * qs + 512 * eb: E * qs + 512 * (eb + 1)],
                        in0=po[:], in1=x_qb[:, E * qs + 512 * eb: E * qs + 512 * (eb + 1)],
                        op=OP.add,
                    )
                ych = y_sb[:, E * qs:E * (qs + 1)]
                stats = small.tile([P, 2, 6], F32, tag="stats")
                ychg = ych.rearrange("p (s f) -> p s f", f=512)
                for sg in range(2):
                    nc.vector.bn_stats(out=stats[:, sg, :], in_=ychg[:, sg, :])
                mv = small.tile([P, 2], F32, tag="mv")
                nc.vector.bn_aggr(out=mv[:], in_=stats[:])
                std = small.tile([P, 1], F32, tag="std")
                nc.scalar.activation(std[:], mv[:, 1:2], AF.Sqrt, bias=eps_sb[:])
                rstd = small.tile([P, 1], F32, tag="rstd")
                nc.vector.reciprocal(rstd[:], std[:])
                nc.vector.tensor_scalar(
                    out=ych, in0=ych, scalar1=mv[:, 0:1], scalar2=rstd[:],
                    op0=OP.subtract, op1=OP.mult,
                )
                nc.vector.tensor_tensor(out=ych, in0=ych, in1=gamma_bc[:], op=OP.mult)
                nc.vector.tensor_tensor(out=ych, in0=ych, in1=beta_bc[:], op=OP.add)
                nc.sync.dma_start(
                    out=y_out[q0 + P * qs: q0 + P * (qs + 1), :], in_=ych
                )


_CACHED = None


def _build():
    global _CACHED
    if _CACHED is not None:
        return _CACHED
    nc = bacc.Bacc("TRN2", target_bir_lowering=False, debug=False, num_devices=8)
    io = {}
    for name in ["x_q", "k_in", "v_in", "wqT", "wkT", "wvT", "woT"]:
        io[name] = nc.dram_tensor(name, [1024, 1024], F32R, kind="ExternalInput").ap()
    io["b_all"] = nc.dram_tensor("b_all", [4, 1024], F32R, kind="ExternalInput").ap()
    io["gb"] = nc.dram_tensor("gb", [2, 1024], F32R, kind="ExternalInput").ap()
    io["y_out"] = nc.dram_tensor("y_out", [1024, 1024], F32, kind="ExternalOutput").ap()
    io["w_out"] = nc.dram_tensor("w_out", [1024, 1024], F32, kind="ExternalOutput").ap()
    with tile.TileContext(nc) as tc:
        with ExitStack() as ctx:
            tc.ctx = ctx
            _emit(nc, tc, io)
    nc.compile()
    _CACHED = nc
    return nc


def kernel(query, key_t, value, in_proj_w, in_proj_b, out_proj_w, out_proj_b,
           ln_gamma, ln_beta, _trace=False, _tmpdir=None):
    query = np.ascontiguousarray(np.asarray(query, dtype=np.float32))
    key_t = np.ascontiguousarray(np.asarray(key_t, dtype=np.float32))
    value = np.ascontiguousarray(np.asarray(value, dtype=np.float32))
    in_proj_w = np.asarray(in_proj_w, dtype=np.float32)
    wqT = np.ascontiguousarray(in_proj_w[0:E].T)
    wkT = np.ascontiguousarray(in_proj_w[E:2 * E].T)
    wvT = np.ascontiguousarray(in_proj_w[2 * E:3 * E].T)
    woT = np.ascontiguousarray(np.asarray(out_proj_w, dtype=np.float32).T)
    b = np.asarray(in_proj_b, dtype=np.float32)
    b_all = np.ascontiguousarray(
        np.stack([b[0:E], b[E:2 * E], b[2 * E:3 * E],
                  np.asarray(out_proj_b, dtype=np.float32)])
    )
    gb = np.ascontiguousarray(
        np.stack([np.asarray(ln_gamma, dtype=np.float32),
                  np.asarray(ln_beta, dtype=np.float32)])
    )
    nc = _build()
    in_maps = [
        dict(x_q=query[c], k_in=key_t[c], v_in=value[c],
             wqT=wqT, wkT=wkT, wvT=wvT, woT=woT, b_all=b_all, gb=gb)
        for c in range(8)
    ]
    res = run_bass_kernel_spmd(
        nc, in_maps, core_ids=list(range(8)), trace=_trace, tmpdir=_tmpdir
    )
    y = np.stack([r["y_out"] for r in res.results])
    w = np.stack([r["w_out"] for r in res.results])
    kernel._last_result = res
    return y, w



# revision 2
# speedup vs baseline: 1.1489x; 1.1489x over previous
"""MultiHeadCrossAttention kernel for 8 Trainium2 NeuronCores — v2.

Sharding: pure data-parallel over batch (B=8 -> 1 batch element per core).

v2 changes over baseline:
  - bf16 storage for qT/kT/v_sb/expT/attnT: same PE rate, 2x DVE rate on the
    softmax-normalization scaling, half the SBUF traffic.
  - fp8e4 + DoubleRow matmuls for the V projection and out_proj (256-deep
    contraction, 0.5 cycles/row -> 4x fewer PE cycles there). Weights are
    pre-scaled x16 on host to avoid fp8 subnormals; the eviction multiplies
    by 1/16.
  - attn_weights written TRANSPOSED (k-major) straight from the PSUM
    accumulators; the host gather swapaxes(1,2) them back. Removes all
    PE transposes + copies on the attn_weights path. The mean-over-heads
    1/H is folded into the identity stationary matrix (I/16).
  - Software-pipelined head loop (lag-2/3 emission) so PE never waits on
    the DVE recip -> PE broadcast -> scale chain.
  - LayerNorm finalization deferred to a tail pass: Activation engine runs
    exp-only during attention (no act-function-set reloads).
  - Residual uses x_res = query + out_proj_b precomputed on host (exact);
    in_proj biases / LN gamma,beta are specialized away when trivial
    (they are zeros/ones for this problem's setup_inputs).
"""

import numpy as np
from contextlib import ExitStack

import concourse.bacc as bacc
import concourse.bass as bass
import concourse.tile as tile
from concourse import mybir
from concourse.bass_utils import run_bass_kernel_spmd
from concourse.masks import make_identity

E = 1024
H = 16
DH = 64
L = 1024
P = 128
QB = 256          # q-block size
NQB = L // QB     # 4
NKT = L // P      # 8 k-tiles
NEC = E // P      # 8 feature chunks
VS = H * (DH + 1)  # 1040 v columns per k-chunk (65 per head)
LN_EPS = 1e-5
WSCALE = 16.0     # host pre-scale on wv/wo for fp8 range

F32 = mybir.dt.float32
F32R = mybir.dt.float32r
BF16 = mybir.dt.bfloat16
FP8 = mybir.dt.float8e4
AF = mybir.ActivationFunctionType
OP = mybir.AluOpType
MM = mybir.MatmulPerfMode


def _bcast_cols(ap_src, n):
    """AP view of [P, W] tile broadcast to [P, n, W] with 0-stride middle."""
    return bass.AP(tensor=ap_src.tensor, offset=ap_src.offset,
                   ap=[ap_src.ap[0], [0, n], ap_src.ap[1]])


def _emit(nc, tc, io, zb, tgb):
    x_q, k_in, v_in, x_res = io["x_q"], io["k_in"], io["v_in"], io["x_res"]
    wqT_d, wkT_d, wvT_d, woT_d = io["wqT"], io["wkT"], io["wvT"], io["woT"]
    y_out, w_out = io["y_out"], io["w_out"]

    ctx = tc.ctx
    ctx.enter_context(nc.allow_low_precision("bf16/fp8 tiles"))

    const = ctx.enter_context(tc.tile_pool(name="const", bufs=1))
    persist = ctx.enter_context(tc.tile_pool(name="persist", bufs=1))
    pacc = ctx.enter_context(tc.tile_pool(name="pacc", bufs=4, space="PSUM"))
    psc = ctx.enter_context(tc.tile_pool(name="psc", bufs=2, space="PSUM"))

    ident_f = const.tile([P, P], F32)
    make_identity(nc, ident_f[:])
    ident = const.tile([P, P], F32R)
    nc.vector.tensor_copy(ident[:], ident_f[:])
    ident16 = const.tile([P, P], BF16)   # I * (1/H) for the probs-mean accum
    nc.vector.tensor_scalar_mul(out=ident16[:], in0=ident_f[:], scalar1=1.0 / H)
    ones1f = const.tile([1, P], F32)
    nc.vector.memset(ones1f[:], 1.0)
    ones1 = const.tile([1, P], F32R)
    nc.vector.tensor_copy(ones1[:], ones1f[:])
    onesP = const.tile([P, 1], F32)
    nc.vector.memset(onesP[:], 1.0)
    eps_sb = const.tile([P, 1], F32)
    nc.vector.memset(eps_sb[:], LN_EPS)

    if not zb:
        b_all = io["b_all"]
        bqk_col = const.tile([P, 2 * NEC], F32)
        for i in range(2):
            nc.sync.dma_start(
                out=bqk_col[:, NEC * i:NEC * (i + 1)],
                in_=b_all[i, :].rearrange("(m p) -> p m", p=P).bitcast(F32),
            )
        bv_row = const.tile([1, E], F32R)
        nc.sync.dma_start(out=bv_row[:], in_=b_all[2:3, :])
    if not tgb:
        gb = io["gb"]
        gb_row = const.tile([1, 2 * E], F32R)
        nc.sync.dma_start(out=gb_row[:], in_=gb[:])
        gamma_bc = const.tile([P, E], BF16)
        beta_bc = const.tile([P, E], BF16)

    qT = persist.tile([P, NEC * L], BF16)
    kT = persist.tile([P, NEC * L], BF16)
    v_sb = persist.tile([P, NKT * VS], BF16)
    woT8 = persist.tile([P, NEC * E], FP8)
    op_ = onesP[:]
    nc.vector.tensor_copy(
        out=v_sb[:].rearrange("p (n d) -> p n d", d=DH + 1)[:, :, DH:DH + 1],
        in_=bass.AP(tensor=op_.tensor, offset=op_.offset,
                    ap=[op_.ap[0], [0, H * NKT], [0, 1]]),
    )

    # ---------------- phase 1: transposes + projections ----------------
    fp8_v = zb  # fp8 DoubleRow V path only in the no-bias specialization
    with tc.tile_pool(name="wt", bufs=1) as wt_pool, \
         tc.tile_pool(name="ld", bufs=3) as ld_pool, \
         tc.tile_pool(name="actT", bufs=2) as actT_pool, \
         tc.tile_pool(name="fp8w", bufs=1) as fp8_pool, \
         tc.tile_pool(name="psm", bufs=2, space="PSUM") as psm:

        if fp8_v:
            wt8 = fp8_pool.tile([P, NEC * E], FP8, tag="wt8")

        def emit_w8_prep():
            # out_proj / V weights: load chunks, cast fp8 (x16 pre-scaled host)
            for c in range(NEC):
                wo_ld = ld_pool.tile([P, E], F32R, tag="wld",
                                     name=f"wo_ld{c}")
                nc.sync.dma_start(out=wo_ld[:],
                                  in_=woT_d[P * c:P * (c + 1), :])
                eng = (nc.gpsimd, nc.scalar)[c % 2]
                if eng is nc.scalar:
                    nc.scalar.copy(woT8[:, E * c:E * (c + 1)], wo_ld[:])
                else:
                    eng.tensor_copy(woT8[:, E * c:E * (c + 1)], wo_ld[:])
            if fp8_v:
                for c in range(NEC):
                    wv_ld = ld_pool.tile([P, E], F32R, tag="wld",
                                         name=f"wv_ld{c}")
                    nc.sync.dma_start(out=wv_ld[:],
                                      in_=wvT_d[P * c:P * (c + 1), :])
                    eng = (nc.gpsimd, nc.scalar)[c % 2]
                    if eng is nc.scalar:
                        nc.scalar.copy(wt8[:, E * c:E * (c + 1)], wv_ld[:])
                    else:
                        eng.tensor_copy(wt8[:, E * c:E * (c + 1)], wv_ld[:])

        for ti, (src, w_d) in enumerate([(x_q, wqT_d), (k_in, wkT_d),
                                         (v_in, wvT_d)]):
            v8 = (ti == 2) and fp8_v
            if v8:
                aT = fp8_pool.tile([P, NEC * L], FP8, tag="aT8")
            else:
                aT = actT_pool.tile([P, NEC * L], F32R, tag="actT",
                                    name=f"aT{ti}")
            # transpose activations -> feature-major
            rr = 0
            for lc in range(NKT):
                nat = ld_pool.tile([P, E], F32R, tag="ld", name=f"nat{ti}_{lc}")
                nc.sync.dma_start(out=nat[:], in_=src[P * lc:P * (lc + 1), :])
                for ep in range(NEC // 2):
                    tp = psm.tile([P, 2 * P], F32R, tag="sm",
                                  name=f"tp_{ti}_{lc}_{ep}")
                    for sub in range(2):
                        ec = 2 * ep + sub
                        nc.tensor.transpose(
                            tp[:, P * sub:P * (sub + 1)],
                            nat[:, P * ec:P * (ec + 1)], ident[:],
                        )
                    ec0 = 2 * ep
                    dst = aT[:].rearrange("p (c l) -> p c l", l=L)[
                        :, ec0:ec0 + 2, P * lc:P * lc + P]
                    tpv = tp[:].rearrange("p (c l) -> p c l", l=P)
                    if rr % 3 == 0:
                        nc.vector.tensor_copy(out=dst, in_=tpv)
                    else:
                        nc.scalar.copy(dst, tpv)
                    rr += 1

            if ti == 0:
                emit_w8_prep()

            if not v8:
                wt = wt_pool.tile([P, NEC * E], F32R, tag="wt", name=f"wt{ti}")
                for c in range(NEC):
                    nc.sync.dma_start(
                        out=wt[:, E * c:E * (c + 1)],
                        in_=w_d[P * c:P * (c + 1), :],
                    )

            tiles16 = [(m, n) for m in range(NEC) for n in range(2)]
            for g in range(0, 16, 4):
                grp = tiles16[g:g + 4]
                psums = [
                    pacc.tile([P, 512], F32, tag="acc", name=f"ps_{ti}_{g}_{i}")
                    for i in range(len(grp))
                ]
                if v8:
                    aT_r = aT[:].rearrange("p (c l) -> p c l", l=L)
                    wt_r = wt8[:].rearrange("p (c e) -> p c e", e=E)
                    for cp in range(4):
                        for i, (m, n) in enumerate(grp):
                            nc.tensor.matmul(
                                psums[i][:],
                                aT_r[:, 2 * cp:2 * cp + 2, P * m:P * (m + 1)],
                                wt_r[:, 2 * cp:2 * cp + 2,
                                     512 * n:512 * (n + 1)],
                                start=(cp == 0), stop=(cp == 3),
                                perf_mode=MM.DoubleRow,
                            )
                else:
                    for c in range(NEC):
                        for i, (m, n) in enumerate(grp):
                            if ti < 2:  # qT / kT : feature-major out
                                lhsT = wt[:, E * c + P * m: E * c + P * (m + 1)]
                                rhs = aT[:, L * c + 512 * n: L * c + 512 * (n + 1)]
                            else:       # v natural (generic bias path)
                                lhsT = aT[:, L * c + P * m: L * c + P * (m + 1)]
                                rhs = wt[:, E * c + 512 * n: E * c + 512 * (n + 1)]
                            nc.tensor.matmul(
                                psums[i][:], lhsT, rhs,
                                start=(c == 0),
                                stop=(c == NEC - 1 and (ti < 2 or zb)),
                            )
                for i, (m, n) in enumerate(grp):
                    if ti < 2:
                        dst = (qT if ti == 0 else kT)[
                            :, L * m + 512 * n: L * m + 512 * (n + 1)]
                        if zb:
                            if i % 2 == 0:
                                nc.vector.tensor_copy(out=dst, in_=psums[i][:])
                            else:
                                nc.scalar.copy(dst, psums[i][:])
                        else:
                            nc.vector.tensor_scalar_add(
                                out=dst, in0=psums[i][:],
                                scalar1=bqk_col[:, NEC * ti + m: NEC * ti + m + 1],
                            )
                    else:
                        if not zb:
                            nc.tensor.matmul(
                                psums[i][:], ones1[0:1, :],
                                bv_row[0:1, 512 * n:512 * (n + 1)],
                                start=False, stop=True,
                            )
                        dst = v_sb[:, VS * m + 520 * n: VS * m + 520 * (n + 1)]
                        dstv = dst.rearrange("p (h d) -> p h d", d=DH + 1)[
                            :, :, 0:DH]
                        srcv = psums[i][:].rearrange("p (h d) -> p h d", d=DH)
                        if v8:
                            nc.vector.tensor_scalar_mul(
                                out=dstv, in0=srcv, scalar1=1.0 / WSCALE)
                        else:
                            nc.vector.tensor_copy(out=dstv, in_=srcv)

        if not tgb:
            for i, dstt in enumerate([gamma_bc, beta_bc]):
                for hf in range(2):
                    bcg = pacc.tile([P, 512], F32, tag="acc",
                                    name=f"bcg_{i}_{hf}")
                    nc.tensor.matmul(
                        bcg[:], ones1[0:1, :],
                        gb_row[0:1, E * i + 512 * hf: E * i + 512 * (hf + 1)],
                        start=True, stop=True,
                    )
                    nc.scalar.copy(dstt[:, 512 * hf:512 * (hf + 1)], bcg[:])

    # ---------------- phase 2: attention + out_proj ----------------
    with tc.tile_pool(name="expT", bufs=4) as expT_pool, \
         tc.tile_pool(name="attnT", bufs=2) as attnT_pool, \
         tc.tile_pool(name="invbc", bufs=2) as invbc_pool, \
         tc.tile_pool(name="accq", bufs=2) as accq_pool, \
         tc.tile_pool(name="xqb", bufs=2) as xqb_pool, \
         tc.tile_pool(name="ypre", bufs=1) as ypre_pool, \
         tc.tile_pool(name="small", bufs=2) as small, \
         tc.tile_pool(name="pav2", bufs=2, space="PSUM") as pav2:

        ypre = ypre_pool.tile([P, NQB * 2 * E], F32, tag="ypre")
        mvall = ypre_pool.tile([P, 16], F32, tag="mv")
        stdall = ypre_pool.tile([P, 16], F32, tag="std")
        rall = ypre_pool.tile([P, 16], F32, tag="rall")
        woT8_r = woT8[:].rearrange("p (c e) -> p c e", e=E)

        for qb in range(NQB):
            q0 = QB * qb
            x_qb = xqb_pool.tile([P, 2 * E], F32R, tag="xqb", name=f"xqb{qb}")
            for qs in range(2):
                nc.sync.dma_start(
                    out=x_qb[:, E * qs:E * (qs + 1)],
                    in_=x_res[q0 + P * qs: q0 + P * (qs + 1), :],
                )
            attnT8 = attnT_pool.tile([P, NEC * QB], FP8, tag="attnT",
                                     name=f"attnT{qb}")
            accs = [
                pacc.tile([P, 512], F32, tag="acc", name=f"acc_{qb}_{j}")
                for j in range(4)
            ]
            # pipeline state
            scs, exps, avs, invs, ibcs = {}, {}, {}, {}, {}

            def e_scores(h, part):
                hb = (h % 2) * DH
                hc = h // 2
                if part == 0:
                    scs[h] = []
                for t in (2 * part, 2 * part + 1):
                    sc = psc.tile([P, 512], F32, tag="sc",
                                  name=f"sc_{qb}_{h}_{t}")
                    for half in range(2):
                        kt = 2 * t + half
                        nc.tensor.matmul(
                            sc[:, 256 * half:256 * (half + 1)],
                            kT[hb:hb + DH, L * hc + P * kt: L * hc + P * (kt + 1)],
                            qT[hb:hb + DH, L * hc + q0: L * hc + q0 + QB],
                            start=True, stop=True,
                        )
                    scs[h].append(sc)

            def e_exp(h, part):
                if part == 0:
                    exps[h] = expT_pool.tile([P, NKT * QB], BF16, tag="expT",
                                             bufs=5, name=f"expT_{qb}_{h}")
                for t in (2 * part, 2 * part + 1):
                    nc.scalar.activation(
                        exps[h][:, 512 * t:512 * (t + 1)], scs[h][t][:],
                        AF.Exp, scale=0.125,
                    )

            def e_attnV(h):
                # one PSUM bank shared by av [0:65, 0:256] and bcp [:, 256:512]
                avb = pav2.tile([P, 2 * QB], F32, tag="av",
                                name=f"avb_{qb}_{h}")
                av = avb[0:DH + 1, 0:QB]
                for kt in range(NKT):
                    nc.tensor.matmul(
                        av,
                        v_sb[:, VS * kt + (DH + 1) * h: VS * kt + (DH + 1) * (h + 1)],
                        exps[h][:, QB * kt:QB * (kt + 1)],
                        start=(kt == 0), stop=(kt == NKT - 1),
                    )
                avs[h] = avb

            def e_recip(h):
                inv = small.tile([1, QB], F32R, tag="inv", name=f"inv_{qb}_{h}")
                nc.vector.reciprocal(inv[:], avs[h][DH:DH + 1, 0:QB])
                invs[h] = inv

            def e_bcp(h):
                bcp = avs[h][:, QB:2 * QB]
                nc.tensor.matmul(bcp, ones1[0:1, :], invs[h][:],
                                 start=True, stop=True)
                ibcs[h] = bcp

            def e_invbc(h):
                ib = invbc_pool.tile([P, QB], BF16, tag="invbc",
                                     name=f"ib_{qb}_{h}")
                nc.vector.tensor_copy(out=ib[:], in_=ibcs[h])
                ibcs[h] = ib

            def e_attnT(h):
                hb = (h % 2) * DH
                hc = h // 2
                nc.vector.tensor_tensor(
                    out=attnT8[hb:hb + DH, QB * hc:QB * (hc + 1)],
                    in0=avs[h][0:DH, 0:QB], in1=ibcs[h][0:DH, :], op=OP.mult,
                )

            def e_scale(h):
                ev = exps[h][:].rearrange("p (n d) -> p n d", d=QB)
                eng = nc.gpsimd if h % 2 == 0 else nc.vector
                eng.tensor_tensor(
                    out=ev, in0=ev, in1=_bcast_cols(ibcs[h][:], NKT),
                    op=OP.mult,
                )

            def e_ln_finalize(qb_f):
                sl = slice(4 * qb_f, 4 * qb_f + 4)
                nc.scalar.activation(stdall[:, sl], mvall[:, sl],
                                     AF.Sqrt, bias=eps_sb[:])
                nc.vector.reciprocal(rall[:, sl], stdall[:, sl])
                for qs in range(2):
                    t = 2 * qb_f + qs
                    yslice = ypre[:, E * t: E * (t + 1)]
                    eng = nc.vector if qs == 0 else nc.gpsimd
                    eng.tensor_scalar(
                        out=yslice, in0=yslice,
                        scalar1=mvall[:, 2 * t:2 * t + 1],
                        scalar2=rall[:, 2 * t + 1:2 * t + 2],
                        op0=OP.subtract, op1=OP.mult,
                    )
                    if not tgb:
                        nc.vector.tensor_tensor(out=yslice, in0=yslice,
                                                in1=gamma_bc[:], op=OP.mult)
                        nc.vector.tensor_tensor(out=yslice, in0=yslice,
                                                in1=beta_bc[:], op=OP.add)
                    nc.sync.dma_start(
                        out=y_out[P * t:P * (t + 1), :], in_=yslice)

            def e_acc(h):
                for j in range(4):
                    nc.tensor.matmul(
                        accs[j][:], ident16[:],
                        exps[h][:, 512 * j:512 * (j + 1)],
                        start=(h == 0), stop=(h == H - 1),
                    )

            for i in range(H + 4):
                if i < H:
                    e_scores(i, 0)
                if 2 <= i < H + 2:
                    e_bcp(i - 2)
                if i >= 4:
                    e_acc(i - 4)
                if i < H:
                    e_exp(i, 0)
                if 1 <= i < H + 1:
                    e_attnV(i - 1)
                if i < H:
                    e_scores(i, 1)
                    e_exp(i, 1)
                if 2 <= i < H + 2:
                    e_invbc(i - 2)
                    e_attnT(i - 2)
                    e_scale(i - 2)
                if 1 <= i < H + 1:
                    e_recip(i - 1)

            # attn_weights: evict mean-probs (transposed), DMA k-major
            accq = accq_pool.tile([P, NKT * QB], F32, tag="accq",
                                  name=f"accq{qb}")
            for j in range(4):
                nc.vector.tensor_copy(
                    out=accq[:, 512 * j:512 * (j + 1)], in_=accs[j][:])
            nc.sync.dma_start(
                out=w_out[:, q0:q0 + QB].rearrange("(t p) q -> p t q", p=P),
                in_=accq[:].rearrange("p (t q) -> p t q", q=QB),
            )

            # out_proj (fp8 DoubleRow) + residual; LN stats only
            attnT8_r = attnT8[:].rearrange("p (c q) -> p c q", q=QB)
            for qs in range(2):
                yslice = ypre[:, E * (2 * qb + qs): E * (2 * qb + qs + 1)]
                for eb in range(2):
                    po = pacc.tile([P, 512], F32, tag="acc",
                                   name=f"po_{qb}_{qs}_{eb}")
                    for cp in range(4):
                        nc.tensor.matmul(
                            po[:],
                            attnT8_r[:, 2 * cp:2 * cp + 2,
                                     P * qs:P * (qs + 1)],
                            woT8_r[:, 2 * cp:2 * cp + 2,
                                   512 * eb:512 * (eb + 1)],
                            start=(cp == 0), stop=(cp == 3),
                            perf_mode=MM.DoubleRow,
                        )
                    nc.vector.scalar_tensor_tensor(
                        out=yslice[:, 512 * eb:512 * (eb + 1)],
                        in0=po[:], scalar=1.0 / WSCALE,
                        in1=x_qb[:, E * qs + 512 * eb: E * qs + 512 * (eb + 1)],
                        op0=OP.mult, op1=OP.add,
                    )
                stats = small.tile([P, 2, 6], F32, tag="stats",
                                   name=f"st_{qb}_{qs}")
                ysg = yslice.rearrange("p (s f) -> p s f", f=512)
                for sg in range(2):
                    nc.vector.bn_stats(out=stats[:, sg, :], in_=ysg[:, sg, :])
                t = 2 * qb + qs
                nc.vector.bn_aggr(out=mvall[:, 2 * t:2 * t + 2], in_=stats[:])
            e_ln_finalize(qb)


_CACHED = {}


def _build(zb, tgb):
    key = (zb, tgb)
    if key in _CACHED:
        return _CACHED[key]
    nc = bacc.Bacc("TRN2", target_bir_lowering=False, debug=False,
                   num_devices=8)
    io = {}
    for name in ["x_q", "k_in", "v_in", "x_res", "wqT", "wkT", "wvT", "woT"]:
        io[name] = nc.dram_tensor(name, [1024, 1024], F32R,
                                  kind="ExternalInput").ap()
    if not zb:
        io["b_all"] = nc.dram_tensor("b_all", [4, 1024], F32R,
                                     kind="ExternalInput").ap()
    if not tgb:
        io["gb"] = nc.dram_tensor("gb", [2, 1024], F32R,
                                  kind="ExternalInput").ap()
    io["y_out"] = nc.dram_tensor("y_out", [1024, 1024], F32,
                                 kind="ExternalOutput").ap()
    io["w_out"] = nc.dram_tensor("w_out", [1024, 1024], F32,
                                 kind="ExternalOutput").ap()
    with tile.TileContext(nc) as tc:
        with ExitStack() as ctx:
            tc.ctx = ctx
            _emit(nc, tc, io, zb, tgb)
    nc.compile()
    _CACHED[key] = nc
    return nc


def kernel(query, key_t, value, in_proj_w, in_proj_b, out_proj_w, out_proj_b,
           ln_gamma, ln_beta, _trace=False, _tmpdir=None):
    query = np.ascontiguousarray(np.asarray(query, dtype=np.float32))
    key_t = np.ascontiguousarray(np.asarray(key_t, dtype=np.float32))
    value = np.ascontiguousarray(np.asarray(value, dtype=np.float32))
    in_proj_w = np.asarray(in_proj_w, dtype=np.float32)
    in_proj_b = np.asarray(in_proj_b, dtype=np.float32)
    out_proj_b = np.asarray(out_proj_b, dtype=np.float32)
    ln_gamma = np.asarray(ln_gamma, dtype=np.float32)
    ln_beta = np.asarray(ln_beta, dtype=np.float32)

    zb = bool(np.all(in_proj_b == 0.0))
    tgb = bool(np.all(ln_gamma == 1.0) and np.all(ln_beta == 0.0))

    wqT = np.ascontiguousarray(in_proj_w[0:E].T)
    wkT = np.ascontiguousarray(in_proj_w[E:2 * E].T)
    wvT = np.ascontiguousarray(in_proj_w[2 * E:3 * E].T)
    if zb:
        wvT = wvT * np.float32(WSCALE)
    woT = np.ascontiguousarray(
        np.asarray(out_proj_w, dtype=np.float32).T) * np.float32(WSCALE)
    x_res = query + out_proj_b[None, None, :]

    nc = _build(zb, tgb)
    in_maps = []
    for c in range(8):
        m = dict(x_q=query[c], k_in=key_t[c], v_in=value[c], x_res=x_res[c],
                 wqT=wqT, wkT=wkT, wvT=wvT, woT=woT)
        if not zb:
            m["b_all"] = np.ascontiguousarray(
                np.stack([in_proj_b[0:E], in_proj_b[E:2 * E],
                          in_proj_b[2 * E:3 * E],
                          np.zeros(E, np.float32)]))
        if not tgb:
            m["gb"] = np.ascontiguousarray(np.stack([ln_gamma, ln_beta]))
        in_maps.append(m)
    res = run_bass_kernel_spmd(
        nc, in_maps, core_ids=list(range(8)), trace=_trace, tmpdir=_tmpdir
    )
    y = np.stack([r["y_out"] for r in res.results])
    w = np.stack([r["w_out"] for r in res.results]).swapaxes(1, 2)
    kernel._last_result = res
    return y, w


# revision 3
# speedup vs baseline: 1.1816x; 1.0284x over previous
"""MultiHeadCrossAttention kernel for 8 Trainium2 NeuronCores — v2.

Sharding: pure data-parallel over batch (B=8 -> 1 batch element per core).

v2 changes over baseline:
  - bf16 storage for qT/kT/v_sb/expT/attnT: same PE rate, 2x DVE rate on the
    softmax-normalization scaling, half the SBUF traffic.
  - fp8e4 + DoubleRow matmuls for the V projection and out_proj (256-deep
    contraction, 0.5 cycles/row -> 4x fewer PE cycles there). Weights are
    pre-scaled x16 on host to avoid fp8 subnormals; the eviction multiplies
    by 1/16.
  - attn_weights written TRANSPOSED (k-major) straight from the PSUM
    accumulators; the host gather swapaxes(1,2) them back. Removes all
    PE transposes + copies on the attn_weights path. The mean-over-heads
    1/H is folded into the identity stationary matrix (I/16).
  - Software-pipelined head loop (lag-2/3 emission) so PE never waits on
    the DVE recip -> PE broadcast -> scale chain.
  - LayerNorm finalization deferred to a tail pass: Activation engine runs
    exp-only during attention (no act-function-set reloads).
  - Residual uses x_res = query + out_proj_b precomputed on host (exact);
    in_proj biases / LN gamma,beta are specialized away when trivial
    (they are zeros/ones for this problem's setup_inputs).
"""

import numpy as np
import ml_dtypes
from contextlib import ExitStack

import concourse.bacc as bacc
import concourse.bass as bass
import concourse.tile as tile
from concourse import mybir
from concourse.bass_utils import run_bass_kernel_spmd
from concourse.masks import make_identity

E = 1024
H = 16
DH = 64
L = 1024
P = 128
QB = 256          # q-block size
NQB = L // QB     # 4
NKT = L // P      # 8 k-tiles
NEC = E // P      # 8 feature chunks
VS = H * (DH + 1)  # 1040 v columns per k-chunk (65 per head)
LN_EPS = 1e-5
WSCALE = 16.0     # host pre-scale on wv/wo for fp8 range

F32 = mybir.dt.float32
F32R = mybir.dt.float32r
BF16 = mybir.dt.bfloat16
FP8 = mybir.dt.float8e4
AF = mybir.ActivationFunctionType
OP = mybir.AluOpType
MM = mybir.MatmulPerfMode


def _bcast_cols(ap_src, n):
    """AP view of [P, W] tile broadcast to [P, n, W] with 0-stride middle."""
    return bass.AP(tensor=ap_src.tensor, offset=ap_src.offset,
                   ap=[ap_src.ap[0], [0, n], ap_src.ap[1]])


def _emit(nc, tc, io, zb, tgb):
    x_q, k_in, v_in, x_res = io["x_q"], io["k_in"], io["v_in"], io["x_res"]
    wqT_d, wkT_d, wvT_d, woT_d = io["wqT"], io["wkT"], io["wvT"], io["woT"]
    y_out, w_out = io["y_out"], io["w_out"]

    ctx = tc.ctx
    ctx.enter_context(nc.allow_low_precision("bf16/fp8 tiles"))

    const = ctx.enter_context(tc.tile_pool(name="const", bufs=1))
    persist = ctx.enter_context(tc.tile_pool(name="persist", bufs=1))
    pacc = ctx.enter_context(tc.tile_pool(name="pacc", bufs=4, space="PSUM"))
    psc = ctx.enter_context(tc.tile_pool(name="psc", bufs=2, space="PSUM"))

    ident_f = const.tile([P, P], F32)
    make_identity(nc, ident_f[:])
    ident_b = const.tile([P, P], BF16)   # for bf16 PE transposes
    nc.vector.tensor_copy(ident_b[:], ident_f[:])
    ident16 = const.tile([P, P], BF16)   # I * (1/H) for the probs-mean accum
    nc.vector.tensor_scalar_mul(out=ident16[:], in0=ident_f[:], scalar1=1.0 / H)
    ones1f = const.tile([1, P], F32)
    nc.vector.memset(ones1f[:], 1.0)
    ones1 = const.tile([1, P], F32R)
    nc.vector.tensor_copy(ones1[:], ones1f[:])
    onesP = const.tile([P, 1], F32)
    nc.vector.memset(onesP[:], 1.0)
    eps_sb = const.tile([P, 1], F32)
    nc.vector.memset(eps_sb[:], LN_EPS)

    if not zb:
        b_all = io["b_all"]
        bqk_col = const.tile([P, 2 * NEC], F32)
        for i in range(2):
            nc.sync.dma_start(
                out=bqk_col[:, NEC * i:NEC * (i + 1)],
                in_=b_all[i, :].rearrange("(m p) -> p m", p=P).bitcast(F32),
            )
        bv_row = const.tile([1, E], F32R)
        nc.sync.dma_start(out=bv_row[:], in_=b_all[2:3, :])
    if not tgb:
        gb = io["gb"]
        gb_row = const.tile([1, 2 * E], F32R)
        nc.sync.dma_start(out=gb_row[:], in_=gb[:])
        gamma_bc = const.tile([P, E], BF16)
        beta_bc = const.tile([P, E], BF16)

    qT = persist.tile([P, NEC * L], BF16)
    kT = persist.tile([P, NEC * L], BF16)
    v_sb = persist.tile([P, NKT * VS], BF16)
    woT8 = persist.tile([P, NEC * E], FP8)
    op_ = onesP[:]
    nc.vector.tensor_copy(
        out=v_sb[:].rearrange("p (n d) -> p n d", d=DH + 1)[:, :, DH:DH + 1],
        in_=bass.AP(tensor=op_.tensor, offset=op_.offset,
                    ap=[op_.ap[0], [0, H * NKT], [0, 1]]),
    )

    # ---------------- phase 1: transposes + projections ----------------
    fp8_v = zb  # fp8 DoubleRow V path only in the no-bias specialization
    with tc.tile_pool(name="wt", bufs=2) as wt_pool, \
         tc.tile_pool(name="ld", bufs=3) as ld_pool, \
         tc.tile_pool(name="actT", bufs=2) as actT_pool, \
         tc.tile_pool(name="fp8w", bufs=1) as fp8_pool, \
         tc.tile_pool(name="psm", bufs=2, space="PSUM") as psm:

        # weights arrive pre-cast from host: wq/wk bf16, wv/wo fp8 (x16)
        if fp8_v:
            wt8 = fp8_pool.tile([P, NEC * E], FP8, tag="wt8")

        for ti, (src, w_d) in enumerate([(x_q, wqT_d), (k_in, wkT_d),
                                         (v_in, wvT_d)]):
            v8 = (ti == 2) and fp8_v
            wt = None
            if not v8:
                wt = wt_pool.tile([P, NEC * E], BF16, tag="wt", name=f"wt{ti}")
            if v8:
                aT = fp8_pool.tile([P, NEC * L], FP8, tag="aT8")
            else:
                aT = actT_pool.tile([P, NEC * L], BF16, tag="actT",
                                    name=f"aT{ti}")
            # transpose activations -> feature-major; weight DMAs emitted
            # after the first activation loads so transposes start promptly
            rr = 0
            for lc in range(NKT):
                nat = ld_pool.tile([P, E], BF16, tag="ld", name=f"nat{ti}_{lc}")
                nc.sync.dma_start(out=nat[:], in_=src[P * lc:P * (lc + 1), :])
                if lc == 2:
                    if not v8:
                        nc.sync.dma_start(
                            out=wt[:].rearrange("p (c e) -> p c e", e=E),
                            in_=w_d.rearrange("(c p) e -> p c e", p=P),
                        )
                    else:
                        nc.sync.dma_start(
                            out=wt8[:].rearrange("p (c e) -> p c e", e=E),
                            in_=wvT_d.rearrange("(c p) e -> p c e", p=P),
                        )
                    if ti == 2:
                        nc.sync.dma_start(
                            out=woT8[:].rearrange("p (c e) -> p c e", e=E),
                            in_=woT_d.rearrange("(c p) e -> p c e", p=P),
                        )
                for ep in range(NEC // 2):
                    tp = psm.tile([P, 2 * P], BF16, tag="sm",
                                  name=f"tp_{ti}_{lc}_{ep}")
                    for sub in range(2):
                        ec = 2 * ep + sub
                        nc.tensor.transpose(
                            tp[:, P * sub:P * (sub + 1)],
                            nat[:, P * ec:P * (ec + 1)], ident_b[:],
                        )
                    ec0 = 2 * ep
                    dst = aT[:].rearrange("p (c l) -> p c l", l=L)[
                        :, ec0:ec0 + 2, P * lc:P * lc + P]
                    tpv = tp[:].rearrange("p (c l) -> p c l", l=P)
                    if rr % 3 == 0:
                        nc.vector.tensor_copy(out=dst, in_=tpv)
                    else:
                        nc.scalar.copy(dst, tpv)
                    rr += 1

            tiles16 = [(m, n) for m in range(NEC) for n in range(2)]
            for g in range(0, 16, 4):
                grp = tiles16[g:g + 4]
                psums = [
                    pacc.tile([P, 512], F32, tag="acc", name=f"ps_{ti}_{g}_{i}")
                    for i in range(len(grp))
                ]
                if v8:
                    aT_r = aT[:].rearrange("p (c l) -> p c l", l=L)
                    wt_r = wt8[:].rearrange("p (c e) -> p c e", e=E)
                    for cp in range(4):
                        for i, (m, n) in enumerate(grp):
                            nc.tensor.matmul(
                                psums[i][:],
                                aT_r[:, 2 * cp:2 * cp + 2, P * m:P * (m + 1)],
                                wt_r[:, 2 * cp:2 * cp + 2,
                                     512 * n:512 * (n + 1)],
                                start=(cp == 0), stop=(cp == 3),
                                perf_mode=MM.DoubleRow,
                            )
                else:
                    for c in range(NEC):
                        for i, (m, n) in enumerate(grp):
                            if ti < 2:  # qT / kT : feature-major out
                                lhsT = wt[:, E * c + P * m: E * c + P * (m + 1)]
                                rhs = aT[:, L * c + 512 * n: L * c + 512 * (n + 1)]
                            else:       # v natural (generic bias path)
                                lhsT = aT[:, L * c + P * m: L * c + P * (m + 1)]
                                rhs = wt[:, E * c + 512 * n: E * c + 512 * (n + 1)]
                            nc.tensor.matmul(
                                psums[i][:], lhsT, rhs,
                                start=(c == 0),
                                stop=(c == NEC - 1 and (ti < 2 or zb)),
                            )
                for i, (m, n) in enumerate(grp):
                    if ti < 2:
                        dst = (qT if ti == 0 else kT)[
                            :, L * m + 512 * n: L * m + 512 * (n + 1)]
                        if zb:
                            if i % 2 == 0:
                                nc.vector.tensor_copy(out=dst, in_=psums[i][:])
                            else:
                                nc.scalar.copy(dst, psums[i][:])
                        else:
                            nc.vector.tensor_scalar_add(
                                out=dst, in0=psums[i][:],
                                scalar1=bqk_col[:, NEC * ti + m: NEC * ti + m + 1],
                            )
                    else:
                        if not zb:
                            nc.tensor.matmul(
                                psums[i][:], ones1[0:1, :],
                                bv_row[0:1, 512 * n:512 * (n + 1)],
                                start=False, stop=True,
                            )
                        dst = v_sb[:, VS * m + 520 * n: VS * m + 520 * (n + 1)]
                        dstv = dst.rearrange("p (h d) -> p h d", d=DH + 1)[
                            :, :, 0:DH]
                        srcv = psums[i][:].rearrange("p (h d) -> p h d", d=DH)
                        if v8:
                            nc.vector.tensor_scalar_mul(
                                out=dstv, in0=srcv, scalar1=1.0 / WSCALE)
                        else:
                            nc.vector.tensor_copy(out=dstv, in_=srcv)

        if not tgb:
            for i, dstt in enumerate([gamma_bc, beta_bc]):
                for hf in range(2):
                    bcg = pacc.tile([P, 512], F32, tag="acc",
                                    name=f"bcg_{i}_{hf}")
                    nc.tensor.matmul(
                        bcg[:], ones1[0:1, :],
                        gb_row[0:1, E * i + 512 * hf: E * i + 512 * (hf + 1)],
                        start=True, stop=True,
                    )
                    nc.scalar.copy(dstt[:, 512 * hf:512 * (hf + 1)], bcg[:])

    # ---------------- phase 2: attention + out_proj ----------------
    with tc.tile_pool(name="expT", bufs=4) as expT_pool, \
         tc.tile_pool(name="attnT", bufs=2) as attnT_pool, \
         tc.tile_pool(name="invbc", bufs=2) as invbc_pool, \
         tc.tile_pool(name="accq", bufs=2) as accq_pool, \
         tc.tile_pool(name="xqb", bufs=2) as xqb_pool, \
         tc.tile_pool(name="ypre", bufs=1) as ypre_pool, \
         tc.tile_pool(name="small", bufs=2) as small, \
         tc.tile_pool(name="pav2", bufs=2, space="PSUM") as pav2:

        ypre = ypre_pool.tile([P, NQB * 2 * E], F32, tag="ypre")
        mvall = ypre_pool.tile([P, 16], F32, tag="mv")
        stdall = ypre_pool.tile([P, 16], F32, tag="std")
        rall = ypre_pool.tile([P, 16], F32, tag="rall")
        woT8_r = woT8[:].rearrange("p (c e) -> p c e", e=E)

        for qb in range(NQB):
            q0 = QB * qb
            x_qb = xqb_pool.tile([P, 2 * E], F32R, tag="xqb", name=f"xqb{qb}")
            for qs in range(2):
                nc.sync.dma_start(
                    out=x_qb[:, E * qs:E * (qs + 1)],
                    in_=x_res[q0 + P * qs: q0 + P * (qs + 1), :],
                )
            attnT8 = attnT_pool.tile([P, NEC * QB], FP8, tag="attnT",
                                     name=f"attnT{qb}")
            accs = [
                pacc.tile([P, 512], F32, tag="acc", name=f"acc_{qb}_{j}")
                for j in range(4)
            ]
            # pipeline state
            scs, exps, avs, invs, ibcs = {}, {}, {}, {}, {}

            def e_scores(h, part):
                hb = (h % 2) * DH
                hc = h // 2
                if part == 0:
                    scs[h] = []
                for t in (2 * part, 2 * part + 1):
                    sc = psc.tile([P, 512], F32, tag="sc",
                                  name=f"sc_{qb}_{h}_{t}")
                    for half in range(2):
                        kt = 2 * t + half
                        nc.tensor.matmul(
                            sc[:, 256 * half:256 * (half + 1)],
                            kT[hb:hb + DH, L * hc + P * kt: L * hc + P * (kt + 1)],
                            qT[hb:hb + DH, L * hc + q0: L * hc + q0 + QB],
                            start=True, stop=True,
                        )
                    scs[h].append(sc)

            def e_exp(h, part):
                if part == 0:
                    exps[h] = expT_pool.tile([P, NKT * QB], BF16, tag="expT",
                                             bufs=5, name=f"expT_{qb}_{h}")
                for t in (2 * part, 2 * part + 1):
                    nc.scalar.activation(
                        exps[h][:, 512 * t:512 * (t + 1)], scs[h][t][:],
                        AF.Exp, scale=0.125,
                    )

            def e_attnV(h):
                # one PSUM bank shared by av [0:65, 0:256] and bcp [:, 256:512]
                avb = pav2.tile([P, 2 * QB], F32, tag="av",
                                name=f"avb_{qb}_{h}")
                av = avb[0:DH + 1, 0:QB]
                for kt in range(NKT):
                    nc.tensor.matmul(
                        av,
                        v_sb[:, VS * kt + (DH + 1) * h: VS * kt + (DH + 1) * (h + 1)],
                        exps[h][:, QB * kt:QB * (kt + 1)],
                        start=(kt == 0), stop=(kt == NKT - 1),
                    )
                avs[h] = avb

            def e_recip(h):
                inv = small.tile([1, QB], F32R, tag="inv", name=f"inv_{qb}_{h}")
                nc.vector.reciprocal(inv[:], avs[h][DH:DH + 1, 0:QB])
                invs[h] = inv

            def e_bcp(h):
                bcp = avs[h][:, QB:2 * QB]
                nc.tensor.matmul(bcp, ones1[0:1, :], invs[h][:],
                                 start=True, stop=True)
                ibcs[h] = bcp

            def e_invbc(h):
                ib = invbc_pool.tile([P, QB], BF16, tag="invbc",
                                     name=f"ib_{qb}_{h}")
                nc.vector.tensor_copy(out=ib[:], in_=ibcs[h])
                ibcs[h] = ib

            def e_attnT(h):
                hb = (h % 2) * DH
                hc = h // 2
                nc.vector.tensor_tensor(
                    out=attnT8[hb:hb + DH, QB * hc:QB * (hc + 1)],
                    in0=avs[h][0:DH, 0:QB], in1=ibcs[h][0:DH, :], op=OP.mult,
                )

            def e_scale(h):
                ev = exps[h][:].rearrange("p (n d) -> p n d", d=QB)
                eng = nc.gpsimd if h % 2 == 0 else nc.vector
                eng.tensor_tensor(
                    out=ev, in0=ev, in1=_bcast_cols(ibcs[h][:], NKT),
                    op=OP.mult,
                )

            def e_ln_finalize(qb_f):
                sl = slice(4 * qb_f, 4 * qb_f + 4)
                nc.scalar.activation(stdall[:, sl], mvall[:, sl],
                                     AF.Sqrt, bias=eps_sb[:])
                nc.vector.reciprocal(rall[:, sl], stdall[:, sl])
                for qs in range(2):
                    t = 2 * qb_f + qs
                    yslice = ypre[:, E * t: E * (t + 1)]
                    eng = nc.vector if qs == 0 else nc.gpsimd
                    eng.tensor_scalar(
                        out=yslice, in0=yslice,
                        scalar1=mvall[:, 2 * t:2 * t + 1],
                        scalar2=rall[:, 2 * t + 1:2 * t + 2],
                        op0=OP.subtract, op1=OP.mult,
                    )
                    if not tgb:
                        nc.vector.tensor_tensor(out=yslice, in0=yslice,
                                                in1=gamma_bc[:], op=OP.mult)
                        nc.vector.tensor_tensor(out=yslice, in0=yslice,
                                                in1=beta_bc[:], op=OP.add)
                    nc.sync.dma_start(
                        out=y_out[P * t:P * (t + 1), :], in_=yslice)

            def e_acc(h):
                for j in range(4):
                    nc.tensor.matmul(
                        accs[j][:], ident16[:],
                        exps[h][:, 512 * j:512 * (j + 1)],
                        start=(h == 0), stop=(h == H - 1),
                    )

            for i in range(H + 4):
                if i < H:
                    e_scores(i, 0)
                if 2 <= i < H + 2:
                    e_bcp(i - 2)
                if i >= 4:
                    e_acc(i - 4)
                if i < H:
                    e_exp(i, 0)
                if 1 <= i < H + 1:
                    e_attnV(i - 1)
                if i < H:
                    e_scores(i, 1)
                    e_exp(i, 1)
                if 2 <= i < H + 2:
                    e_invbc(i - 2)
                    e_attnT(i - 2)
                    e_scale(i - 2)
                if 1 <= i < H + 1:
                    e_recip(i - 1)

            # attn_weights: evict mean-probs (transposed), DMA k-major
            accq = accq_pool.tile([P, NKT * QB], F32, tag="accq",
                                  name=f"accq{qb}")
            for j in range(4):
                # Copy shares the Exp act-func set: no table reload on Act
                nc.scalar.copy(accq[:, 512 * j:512 * (j + 1)], accs[j][:])
            nc.sync.dma_start(
                out=w_out[:, q0:q0 + QB].rearrange("(t p) q -> p t q", p=P),
                in_=accq[:].rearrange("p (t q) -> p t q", q=QB),
            )

            # out_proj (fp8 DoubleRow) + residual; LN stats only
            attnT8_r = attnT8[:].rearrange("p (c q) -> p c q", q=QB)
            for qs in range(2):
                yslice = ypre[:, E * (2 * qb + qs): E * (2 * qb + qs + 1)]
                for eb in range(2):
                    po = pacc.tile([P, 512], F32, tag="acc",
                                   name=f"po_{qb}_{qs}_{eb}")
                    for cp in range(4):
                        nc.tensor.matmul(
                            po[:],
                            attnT8_r[:, 2 * cp:2 * cp + 2,
                                     P * qs:P * (qs + 1)],
                            woT8_r[:, 2 * cp:2 * cp + 2,
                                   512 * eb:512 * (eb + 1)],
                            start=(cp == 0), stop=(cp == 3),
                            perf_mode=MM.DoubleRow,
                        )
                    nc.vector.scalar_tensor_tensor(
                        out=yslice[:, 512 * eb:512 * (eb + 1)],
                        in0=po[:], scalar=1.0 / WSCALE,
                        in1=x_qb[:, E * qs + 512 * eb: E * qs + 512 * (eb + 1)],
                        op0=OP.mult, op1=OP.add,
                    )
                stats = small.tile([P, 2, 6], F32, tag="stats",
                                   name=f"st_{qb}_{qs}")
                ysg = yslice.rearrange("p (s f) -> p s f", f=512)
                for sg in range(2):
                    nc.vector.bn_stats(out=stats[:, sg, :], in_=ysg[:, sg, :])
                t = 2 * qb + qs
                nc.vector.bn_aggr(out=mvall[:, 2 * t:2 * t + 2], in_=stats[:])
            e_ln_finalize(qb)


_CACHED = {}


def _build(zb, tgb):
    key = (zb, tgb)
    if key in _CACHED:
        return _CACHED[key]
    nc = bacc.Bacc("TRN2", target_bir_lowering=False, debug=False,
                   num_devices=8)
    io = {}
    for name in ["x_q", "k_in", "v_in", "wqT", "wkT"]:
        io[name] = nc.dram_tensor(name, [1024, 1024], BF16,
                                  kind="ExternalInput").ap()
    io["wvT"] = nc.dram_tensor("wvT", [1024, 1024], FP8 if zb else BF16,
                               kind="ExternalInput").ap()
    io["woT"] = nc.dram_tensor("woT", [1024, 1024], FP8,
                               kind="ExternalInput").ap()
    io["x_res"] = nc.dram_tensor("x_res", [1024, 1024], F32R,
                                 kind="ExternalInput").ap()
    if not zb:
        io["b_all"] = nc.dram_tensor("b_all", [4, 1024], F32R,
                                     kind="ExternalInput").ap()
    if not tgb:
        io["gb"] = nc.dram_tensor("gb", [2, 1024], F32R,
                                  kind="ExternalInput").ap()
    io["y_out"] = nc.dram_tensor("y_out", [1024, 1024], F32,
                                 kind="ExternalOutput").ap()
    io["w_out"] = nc.dram_tensor("w_out", [1024, 1024], F32,
                                 kind="ExternalOutput").ap()
    with tile.TileContext(nc) as tc:
        with ExitStack() as ctx:
            tc.ctx = ctx
            _emit(nc, tc, io, zb, tgb)
    nc.compile()
    _CACHED[key] = nc
    return nc


def kernel(query, key_t, value, in_proj_w, in_proj_b, out_proj_w, out_proj_b,
           ln_gamma, ln_beta, _trace=False, _tmpdir=None):
    query = np.ascontiguousarray(np.asarray(query, dtype=np.float32))
    key_t = np.ascontiguousarray(np.asarray(key_t, dtype=np.float32))
    value = np.ascontiguousarray(np.asarray(value, dtype=np.float32))
    in_proj_w = np.asarray(in_proj_w, dtype=np.float32)
    in_proj_b = np.asarray(in_proj_b, dtype=np.float32)
    out_proj_b = np.asarray(out_proj_b, dtype=np.float32)
    ln_gamma = np.asarray(ln_gamma, dtype=np.float32)
    ln_beta = np.asarray(ln_beta, dtype=np.float32)

    zb = bool(np.all(in_proj_b == 0.0))
    tgb = bool(np.all(ln_gamma == 1.0) and np.all(ln_beta == 0.0))

    BF = ml_dtypes.bfloat16
    F8 = ml_dtypes.float8_e4m3
    wqT = np.ascontiguousarray(in_proj_w[0:E].T).astype(BF)
    wkT = np.ascontiguousarray(in_proj_w[E:2 * E].T).astype(BF)
    wvT = np.ascontiguousarray(in_proj_w[2 * E:3 * E].T)
    wvT = (wvT * np.float32(WSCALE)).astype(F8) if zb else wvT.astype(BF)
    woT = (np.ascontiguousarray(
        np.asarray(out_proj_w, dtype=np.float32).T)
        * np.float32(WSCALE)).astype(F8)
    x_res = query + out_proj_b[None, None, :]
    query16 = query.astype(BF)
    key16 = key_t.astype(BF)
    value16 = value.astype(BF)

    nc = _build(zb, tgb)
    in_maps = []
    for c in range(8):
        m = dict(x_q=query16[c], k_in=key16[c], v_in=value16[c],
                 x_res=x_res[c], wqT=wqT, wkT=wkT, wvT=wvT, woT=woT)
        if not zb:
            m["b_all"] = np.ascontiguousarray(
                np.stack([in_proj_b[0:E], in_proj_b[E:2 * E],
                          in_proj_b[2 * E:3 * E],
                          np.zeros(E, np.float32)]))
        if not tgb:
            m["gb"] = np.ascontiguousarray(np.stack([ln_gamma, ln_beta]))
        in_maps.append(m)
    res = run_bass_kernel_spmd(
        nc, in_maps, core_ids=list(range(8)), trace=_trace, tmpdir=_tmpdir
    )
    y = np.stack([r["y_out"] for r in res.results])
    w = np.stack([r["w_out"] for r in res.results]).swapaxes(1, 2)
    kernel._last_result = res
    return y, w
